# revision 24
# baseline (speedup 1.0000x reference)
"""Bass/Trainium2 kernel for nn_BilinearInteractionLayer.

Computes, for all field pairs (i, j) with i < j (P = C(32,2) = 496 pairs):
    out[b, p, :] = (emb[b, i_p, :] @ W[p].T) * emb[b, j_p, :]
with emb [2048, 32, 64] fp32 and W [496, 64, 64] fp32.

Strategy: data-parallel over batch across 8 cores (B=256 per core, two
128-row b-chunks), W replicated. Pairs are grouped by left field i so each
matmul computes proj[b, (j, e)] = X_f[b, :] @ Wcat_f.T with the 128-row
batch chunk as the PE stationary operand and the stacked pair weights
streaming. Fields are split into two "planes" (0..8 on SBUF partitions
0:64, 9..30 on partitions 64:128) so the packed weight tensor uses all 128
partitions and the two planes' matmuls run on distinct PE row-groups
(K=64, tile_position (0,0)/(64,0)) and overlap.

Eviction pipeline (PSUM fp32 -> fp16 stage in SBUF, fused * v_j):
  - path S (~1/4 of tiles): one DVE scalar_tensor_tensor straight from
    PSUM: stage = (psum * 1.0) * v_j  (1x mode, single pass)
  - path C (rest): ACT copies PSUM fp32 -> SBUF fp16, then the DVE
    multiply is all-16-bit SBUF->SBUF and runs in 2x_1P mode.
This balances ACT and DVE at ~48us each, under the ~62us HBM-DMA floor
(22.3 MB per core at ~360 GB/s), which is the real roofline.

Output is written to HBM as fp16 (halving the dominant DMA stream) and
upcast to fp32 on the host during the gather. Matmul operands are fp16
(rel err ~3e-4 with fp32 PSUM accumulation); v_j is fp16 as well.

Output DMAs alternate between the two HWDGE rings (SP + ACT) once the
input loads have drained from the ACT ring.
"""

import sys

sys.path.insert(0, "/opt/trn_rl_repo")

from contextlib import ExitStack
from itertools import combinations

import numpy as np

import concourse.bass as bass
import concourse.tile as tile
from concourse import bacc, bass_utils, mybir
from concourse._compat import with_exitstack

NUM_FIELDS = 32
EMB_DIM = 64
BATCH = 2048
N_CORES = 8
B_CORE = BATCH // N_CORES          # 256
N_BCHUNK = B_CORE // 128           # 2
PAIRS = list(combinations(range(NUM_FIELDS), 2))
P_TOTAL = len(PAIRS)               # 496

# OFF[f] = global pair index of first pair (f, f+1)
OFF = [0] * NUM_FIELDS
for _f in range(1, NUM_FIELDS):
    OFF[_f] = OFF[_f - 1] + (NUM_FIELDS - _f)

# Plane split: fields 0..8 (243 pairs) on partitions 0:64, fields 9..30
# (253 pairs) on partitions 64:128.
PLANE_FIELDS = (list(range(0, 9)), list(range(9, 31)))
PLANE_P0 = (0, OFF[9])                       # 0, 243
PLANE_NP = (OFF[9] - 0, P_TOTAL - OFF[9])    # 243, 253
WT_COLS = max(PLANE_NP) * EMB_DIM            # 16192

MM_N = 512            # max cols per matmul (one PSUM bank, fp32)
PSUM_COLS = 1024      # psum tile width (2 banks)
STAGE_COLS = 4096     # stage tile width
WT_CHUNK = 4096       # wt DMA chunk (pair-aligned: 64 pairs)
WT_NCHUNK = (WT_COLS + WT_CHUNK - 1) // WT_CHUNK  # 4
N_LO_FIELDS = len(PLANE_FIELDS[0])           # 9
N_HI_FIELDS = len(PLANE_FIELDS[1])           # 22
EMBT_LO_COLS = N_LO_FIELDS * 128             # 1152
EMBT_HI_COLS = N_HI_FIELDS * 128             # 2816
# Per-chunk emb data ships as ONE full-width [128, EMBX_COLS] tensor:
# cols 0:2816 hold the transposed lhsT planes (partitions 0:64 = fields 0-8,
# partitions 64:128 = fields 9-30), cols 2816:4864 hold the natural-layout
# embn. 64-partition DMAs only engage half the SDMA ports; this keeps the
# input stream full-width and cuts the dma_start issue count.
EMBN_OFF = EMBT_HI_COLS                      # 2816
EMBX_COLS = EMBT_HI_COLS + NUM_FIELDS * EMB_DIM  # 4864
EMB_DT = mybir.dt.float16
EMB_NP = np.float16
OUT_DT = mybir.dt.float16
OUT_NP = np.float16

SYNC_ONLY_FLUSHES = 10  # first N output DMAs go on the SP ring only
FLUSH_COLS = 2048       # flush a group's stage to HBM in ~this many cols
GP_MIN_FD = 512         # gpsimd only gets tiles at least this wide

# Static greedy engine-load balancing for the eviction paths (costs in ns,
# fitted from HW traces):
#   stt     : DVE single-pass from PSUM   dve += 125 + FD*1.042
#   act+dve : ACT copy, DVE 16-bit mul    act += 190 + FD*0.833 ; dve += 60 + FD*0.542
#   act+gp  : ACT copy, GpSimd 16-bit mul act += 190 + FD*0.833 ; gp  += 550 + FD*1.65
# ACT starts biased (+2.5us): its queue runs the input dma_start issues and
# the activation-table load before the first copy can run; GpSimd starts
# biased (+1us) for its preamble memsets.


def _evict_plan(entries_fds):
    """Assign each eviction (by FD) a path minimizing the running max load.

    Cross-engine contention terms (fitted from HW traces): DVE stt reads
    PSUM, serializing against ACT's PSUM reads on bank overlap (charge ACT
    0.25*FD per stt); GpSimd shares its SBUF port with DVE (charge DVE
    0.2*FD per gp multiply)."""
    dve, act, gp = 0.0, 1500.0, 1000.0
    plan = []
    for fd in entries_fds:
        cands = [
            ("stt", max(dve + 125 + fd * 1.35, act + fd * 0.25, gp)),
            ("act_dve", max(dve + 60 + fd * 0.542, act + 170 + fd * 0.833, gp)),
        ]
        if fd >= GP_MIN_FD and gp + 550 + fd * 1.8 < 26000:
            cands.append(
                (
                    "act_gp",
                    max(
                        dve + fd * 0.2,
                        act + 170 + fd * 0.833,
                        gp + 550 + fd * 1.8,
                    ),
                )
            )
        path = min(cands, key=lambda x: x[1])[0]
        if path == "stt":
            dve += 125 + fd * 1.35
            act += fd * 0.25
        elif path == "act_dve":
            act += 170 + fd * 0.833
            dve += 60 + fd * 0.542
        else:
            act += 170 + fd * 0.833
            gp += 550 + fd * 1.8
            dve += fd * 0.2
        plan.append(path)
    return plan


def _field_cols(f):
    return (NUM_FIELDS - 1 - f) * EMB_DIM


def _field_groups(plane):
    """Group consecutive fields of a plane so each group's output columns fit
    in one stage tile (one output DMA per group per b-chunk)."""
    groups = []
    cur, cur_cols = [], 0
    for f in PLANE_FIELDS[plane]:
        cols = _field_cols(f)
        if cur and cur_cols + cols > STAGE_COLS:
            groups.append(cur)
            cur, cur_cols = [], 0
        cur.append(f)
        cur_cols += cols
    if cur:
        groups.append(cur)
    return groups


def _plane_entries(plane):
    """Flatten a plane's work into psum-tile entries, in program order.

    Entry: dict(plane, c, f, group_key, stage_off, ck0, cols, mms,
    first_in_group, flush). mms are (abs_col, tile_k0, n) splits at
    PSUM-bank (512) and wt-chunk (4096) boundaries. flush, when set, is
    (pair0, npairs, stage_lo, stage_hi): after this entry's eviction the
    stage slice [stage_lo:stage_hi] (= global pairs [pair0, pair0+npairs))
    is DMAed to HBM. Groups flush roughly every FLUSH_COLS columns (at
    field boundaries) so output DMA starts early and the tail is short."""
    entries = []
    groups = _field_groups(plane)
    for c in range(N_BCHUNK):
        for gi, fields in enumerate(groups):
            stage_off = 0
            flush_lo = 0          # stage col where the pending flush begins
            flushed_pairs = OFF[fields[0]]
            for fi, f in enumerate(fields):
                cols = _field_cols(f)
                p_local = OFF[f] - PLANE_P0[plane]
                col0 = p_local * EMB_DIM
                field_end = stage_off + cols
                last_field = fi == len(fields) - 1
                for ck0 in range(0, cols, PSUM_COLS):
                    ccols = min(PSUM_COLS, cols - ck0)
                    mms = []
                    k0 = 0
                    while k0 < ccols:
                        abs_col = col0 + ck0 + k0
                        n = min(MM_N, ccols - k0)
                        # don't cross a wt DMA-chunk boundary (separate tiles)
                        chunk_end = ((abs_col // WT_CHUNK) + 1) * WT_CHUNK
                        n = min(n, chunk_end - abs_col)
                        # don't cross a PSUM bank boundary (512 fp32 cols)
                        n = min(n, MM_N - (k0 % MM_N))
                        mms.append((abs_col, k0, n))
                        k0 += n
                    last_of_field = ck0 + ccols >= cols
                    flush = None
                    if last_of_field and (
                        last_field or field_end - flush_lo >= FLUSH_COLS
                    ):
                        npairs = (field_end - flush_lo) // EMB_DIM
                        flush = (flushed_pairs, npairs, flush_lo, field_end)
                        flushed_pairs += npairs
                        flush_lo = field_end
                    entries.append(
                        dict(
                            plane=plane,
                            c=c,
                            f=f,
                            group_key=(plane, c, gi),
                            stage_off=stage_off + ck0,
                            cols=ccols,
                            ck0=ck0,
                            mms=mms,
                            first_in_group=(fi == 0 and ck0 == 0),
                            flush=flush,
                            last_in_group=(last_field and ck0 + ccols >= cols),
                        )
                    )
                stage_off += cols
    return entries


@with_exitstack
def _bilinear_kernel(
    ctx: ExitStack,
    tc: "tile.TileContext",
    out_ap: bass.AP,
    wt_aps,
    embt_lo_aps,
    embt_hi_aps,
    embn_aps,
):
    nc = tc.nc

    wt_pool = ctx.enter_context(tc.tile_pool(name="wt", bufs=WT_NCHUNK))
    embt_pool = ctx.enter_context(tc.tile_pool(name="embt", bufs=N_BCHUNK))
    embn_pool = ctx.enter_context(tc.tile_pool(name="embn", bufs=N_BCHUNK))
    psum_pool = ctx.enter_context(tc.tile_pool(name="psum", bufs=4, space="PSUM"))
    proj_pool = ctx.enter_context(tc.tile_pool(name="proj", bufs=6))
    stage_pool = ctx.enter_context(tc.tile_pool(name="stage", bufs=8))

    embt_tiles, embn_tiles = [], []
    for c in range(N_BCHUNK):
        et = embt_pool.tile(
            [128, EMBT_HI_COLS], EMB_DT, tag="embt", name=f"embt{c}"
        )
        embt_tiles.append(et)
        en = embn_pool.tile(
            [128, NUM_FIELDS * EMB_DIM], EMB_DT, tag="embn", name=f"embn{c}"
        )
        embn_tiles.append(en)
    wt_tiles = []
    for k in range(WT_NCHUNK):
        cols = min(WT_CHUNK, WT_COLS - k * WT_CHUNK)
        t = wt_pool.tile([128, cols], EMB_DT, tag="wt", name=f"wtt{k}")
        wt_tiles.append(t)

    # Input loads: wt chunk 0 on the SP ring (parallel with the ACT ring),
    # everything else on the ACT ring in first-needed order (wt2/3 are
    # consumed before the b-chunk-1 emb data). Few, large DMAs: each
    # dma_start costs ~0.6us of issuing-engine queue time. wt0 is split so
    # its first piece (which gates the very first matmuls) lands ~3us
    # earlier than the full 1MB chunk would.
    nc.sync.dma_start(wt_tiles[0][:], wt_aps[0][:])
    nc.scalar.dma_start(embt_tiles[0][0:64, 0:EMBT_LO_COLS], embt_lo_aps[0][:])
    nc.scalar.dma_start(embt_tiles[0][64:128, 0:EMBT_HI_COLS], embt_hi_aps[0][:])
    nc.scalar.dma_start(embn_tiles[0][:], embn_aps[0][:])
    nc.scalar.dma_start(wt_tiles[1][:], wt_aps[1][:])
    nc.scalar.dma_start(embt_tiles[1][0:64, 0:EMBT_LO_COLS], embt_lo_aps[1][:])
    nc.scalar.dma_start(embt_tiles[1][64:128, 0:EMBT_HI_COLS], embt_hi_aps[1][:])
    nc.scalar.dma_start(embn_tiles[1][:], embn_aps[1][:])
    nc.scalar.dma_start(wt_tiles[2][:], wt_aps[2][:])
    nc.scalar.dma_start(wt_tiles[3][:], wt_aps[3][:])

    def lhsT_of(e):
        r0 = 64 * e["plane"]
        if e["plane"] == 0:
            fslot = e["f"]
        else:
            fslot = e["f"] - PLANE_FIELDS[1][0]
        return embt_tiles[e["c"]][r0 : r0 + 64, fslot * 128 : fslot * 128 + 128]

    queues = [_plane_entries(0), _plane_entries(1)]

    # Interleave the two planes' entries 1:1 (plane-1 tail runs alone), then
    # compute the static engine-balanced eviction plan in that order.
    order = []
    idx = [0, 0]
    while idx[0] < len(queues[0]) or idx[1] < len(queues[1]):
        for p in (0, 1):
            if idx[p] < len(queues[p]):
                order.append(queues[p][idx[p]])
                idx[p] += 1
    plan = _evict_plan([e["cols"] for e in order])

    stages = {}          # group_key -> stage tile
    flush_count = [0]

    def emit(entry_list):
        for e in entry_list:
            if e["first_in_group"]:
                stages[e["group_key"]] = stage_pool.tile(
                    [128, STAGE_COLS], OUT_DT, tag="stage", name="stg"
                )
            e["ps"] = psum_pool.tile(
                [128, PSUM_COLS], mybir.dt.float32, tag="ps", name="ps"
            )
        # zip matmuls across entries (planes) for PE row-group overlap
        maxmm = max(len(e["mms"]) for e in entry_list)
        for k in range(maxmm):
            for e in entry_list:
                if k < len(e["mms"]):
                    abs_col, pk0, n = e["mms"][k]
                    r0 = 64 * e["plane"]
                    wtt = wt_tiles[abs_col // WT_CHUNK]
                    wc = abs_col % WT_CHUNK
                    nc.tensor.matmul(
                        e["ps"][:, pk0 : pk0 + n],
                        lhsT_of(e),
                        wtt[r0 : r0 + 64, wc : wc + n],
                        start=True,
                        stop=True,
                    )
        for e in entry_list:
            st = stages[e["group_key"]]
            cols = e["cols"]
            e0 = (e["f"] + 1) * EMB_DIM + e["ck0"]
            dst = st[:, e["stage_off"] : e["stage_off"] + cols]
            in1 = embn_tiles[e["c"]][:, e0 : e0 + cols]
            path = e["path"]
            if path == "stt":
                # single-pass DVE: stage = (psum * 1) * v_j
                nc.vector.scalar_tensor_tensor(
                    dst,
                    e["ps"][:, 0:cols],
                    1.0,
                    in1,
                    mybir.AluOpType.mult,
                    mybir.AluOpType.mult,
                )
            else:
                tmp = proj_pool.tile(
                    [128, PSUM_COLS], EMB_DT, tag="proj", name="proj"
                )
                nc.scalar.copy(tmp[:, 0:cols], e["ps"][:, 0:cols])
                eng = nc.vector if path == "act_dve" else nc.gpsimd
                eng.tensor_mul(dst, tmp[:, 0:cols], in1)
            if e["flush"] is not None:
                p0, npair, slo, shi = e["flush"]
                c = e["c"]
                flush_count[0] += 1
                # all output DMAs issue from the otherwise-idle SP engine
                # (a dma_start costs ~0.6us of issuing-engine queue time)
                nc.sync.dma_start(
                    out_ap[c * 128 : (c + 1) * 128, p0 : p0 + npair, :],
                    st[:, slo:shi],
                )
            if e["last_in_group"]:
                del stages[e["group_key"]]

    k = 0
    while k < len(order):
        batch = [order[k]]
        if k + 1 < len(order) and order[k + 1]["plane"] != order[k]["plane"]:
            batch.append(order[k + 1])
        for i, e in enumerate(batch):
            e["path"] = plan[k + i]
        emit(batch)
        k += len(batch)


_CACHE = {}


def _get_program():
    if "nc" not in _CACHE:
        nc = bacc.Bacc(
            "TRN2", target_bir_lowering=False, debug=False, num_devices=N_CORES
        )
        wt_aps = []
        for k in range(WT_NCHUNK):
            cols = min(WT_CHUNK, WT_COLS - k * WT_CHUNK)
            wt_aps.append(
                nc.dram_tensor(
                    f"wt{k}", [128, cols], EMB_DT, kind="ExternalInput"
                ).ap()
            )
        embt_lo_aps = [
            nc.dram_tensor(
                f"embtl{c}", [64, EMBT_LO_COLS], EMB_DT, kind="ExternalInput"
            ).ap()
            for c in range(N_BCHUNK)
        ]
        embt_hi_aps = [
            nc.dram_tensor(
                f"embth{c}", [64, EMBT_HI_COLS], EMB_DT, kind="ExternalInput"
            ).ap()
            for c in range(N_BCHUNK)
        ]
        embn_aps = [
            nc.dram_tensor(
                f"embn{c}", [128, NUM_FIELDS * EMB_DIM], EMB_DT,
                kind="ExternalInput",
            ).ap()
            for c in range(N_BCHUNK)
        ]
        out_ap = nc.dram_tensor(
            "out", [B_CORE, P_TOTAL, EMB_DIM], OUT_DT, kind="ExternalOutput"
        ).ap()
        with tile.TileContext(nc) as tc:
            _bilinear_kernel(
                tc, out_ap, wt_aps, embt_lo_aps, embt_hi_aps, embn_aps
            )
        nc.compile()
        _CACHE["nc"] = nc
    return _CACHE["nc"]


def _pack_wt(W: np.ndarray):
    """W [496, 64, 64] fp32 -> WT_NCHUNK chunks of [128, <=4096] fp16 with
    wt[64*plane + d, p_local*64 + e] = W[p, e, d]."""
    Wh = W.astype(EMB_NP)
    full = np.zeros((128, WT_COLS), dtype=EMB_NP)
    for plane in (0, 1):
        p0, npair = PLANE_P0[plane], PLANE_NP[plane]
        blk = Wh[p0 : p0 + npair].transpose(2, 0, 1).reshape(EMB_DIM, npair * EMB_DIM)
        full[64 * plane : 64 * plane + EMB_DIM, : npair * EMB_DIM] = blk
    return [
        np.ascontiguousarray(full[:, k * WT_CHUNK : min((k + 1) * WT_CHUNK, WT_COLS)])
        for k in range(WT_NCHUNK)
    ]


def _pack_core_inputs(emb_shard: np.ndarray):
    """emb_shard [256, 32, 64] fp32 -> per-b-chunk (embt_lo [64, 1152],
    embt_hi [64, 2816], embn [128, 2048]) fp16 arrays; embt col = fslot*128 + b."""
    lo, hi, nat = [], [], []
    for c in range(N_BCHUNK):
        chunk = emb_shard[c * 128 : (c + 1) * 128]  # [128, 32, 64]
        et = chunk.transpose(2, 1, 0).astype(EMB_NP)  # [64, 32, 128]
        lo.append(
            np.ascontiguousarray(
                et[:, : N_LO_FIELDS, :].reshape(EMB_DIM, EMBT_LO_COLS)
            )
        )
        hi.append(
            np.ascontiguousarray(
                et[:, N_LO_FIELDS : N_LO_FIELDS + N_HI_FIELDS, :].reshape(
                    EMB_DIM, EMBT_HI_COLS
                )
            )
        )
        nat.append(
            np.ascontiguousarray(
                chunk.reshape(128, NUM_FIELDS * EMB_DIM).astype(EMB_NP)
            )
        )
    return lo, hi, nat


def build_in_maps(feature_emb: np.ndarray, W: np.ndarray):
    wt_chunks = _pack_wt(np.asarray(W))
    emb = np.asarray(feature_emb, dtype=np.float32)
    in_maps = []
    for i in range(N_CORES):
        lo, hi, nat = _pack_core_inputs(emb[i * B_CORE : (i + 1) * B_CORE])
        m = {}
        for c in range(N_BCHUNK):
            m[f"embtl{c}"] = lo[c]
            m[f"embth{c}"] = hi[c]
            m[f"embn{c}"] = nat[c]
        for k, w in enumerate(wt_chunks):
            m[f"wt{k}"] = w
        in_maps.append(m)
    return in_maps


def run(feature_emb: np.ndarray, W: np.ndarray, trace: bool = False, tmpdir=None):
    """Returns (out [2048, 496, 64] fp32, BassKernelResults)."""
    nc = _get_program()
    in_maps = build_in_maps(feature_emb, W)
    res = bass_utils.run_bass_kernel_spmd(
        nc, in_maps, core_ids=list(range(N_CORES)), trace=trace, tmpdir=tmpdir
    )
    out = np.concatenate(
        [res.results[i]["out"] for i in range(N_CORES)], axis=0
    ).astype(np.float32)
    return out, res


def kernel(feature_emb: np.ndarray, W: np.ndarray) -> np.ndarray:
    out, _ = run(feature_emb, W)
    return out
